# revision 96
# baseline (speedup 1.0000x reference)
"""GQA prefill kernel for 8 Trainium2 NeuronCores (software-pipelined).

Problem: B=2, T=2048, C=2048, H=32 q-heads, HKV=8 kv-heads, DH=64,
causal attention with RoPE, torch-Linear-style projections.

Sharding: core = b*4 + g over (batch b in 0..1, head-group g in 0..3).
Each core owns 8 q-heads / 2 kv-heads of one batch element:
  - Wq column-shard   -> qT   [512, T]  (features on partitions)
  - Wkv column-shard  -> kT,vT[128, T]
  - Wo row-shard      -> partial output [T, C]; host sums 4 partials/batch
    and divides by WSCALE.

Design:
  - Error-compensated fp8 DoubleRow matmuls for the deep-contraction
    projections (QKV and output): every operand X is host/on-chip split
    into X8 = fp8(X) plus a residual dX8 = fp8(X - X8), with weights
    pre-scaled by 64 so both parts sit in e4m3's normal range.  The
    product is X8.W8 (chunk-PAIRS per DoubleRow instruction) plus one
    DoubleRow instruction per chunk computing dX8.W8 + X8.dW8 via the
    pair dimension, all accumulated in one f32 PSUM group.  At the cost
    model's 0.5 cycles/row this is 0.75x the bf16 matmul cost with
    ~0.1% error.  The 64x weight scale cancels for free: /64 is folded
    into the cos/sin RoPE tables (q, k), a 64.0 denominator column (v),
    and the host-side reduction (out).  Residual halves are interleaved
    with main halves inside single SBUF tiles so each residual
    instruction's (dW8[c].x8[c] + W8[c].dx8[c]) pair is two strided APs
    of the same tiles.
  - Attention stays bf16 (scores contract over DH=64 only, and fp8
    probs/V would put ~2.5% noise straight onto the output).
  - One software pipeline over j (512-wide q/t blocks): attention of
    block j interleaves the QKV chains of block j+1 as PE "fill" units;
    x for block j+2 prefetches on the idle SP queue (2 DMAs per block
    into one 16KB tile).  Block 0 defers its q2/q3 chains into the
    first two attention groups (drained PAST the generator tail -- a
    chain's RoPE is only emitted by the next() after its last unit).
  - The exp for a non-diagonal key tile is ONE wide ACT instruction
    over both heads of the group's head pair ([128, 1024] across two
    PSUM banks); matmuls must stay within one PSUM bank (512 f32).
  - attn@V accumulates unnormalized with a 64-valued denominator column
    riding as row 64; at group end the accumulator is evacuated to SBUF
    split across ACT and DVE (freeing PSUM for the next group), then
    reciprocal/broadcast/normalize run off the copy, and the result is
    split to a8/da8 (Pool copy + Pool subtract) for the output
    projection.
  - Output projection epilogue interleaves cb-group pairs on the halves
    of wide sc-tag PSUM tiles and alternates PSUM evacuation between
    ACT and DVE, with per-512-column stores for the final tiles.
  - RoPE without partition-shift DMAs: the host lays out the dh axis so
    rotate-half partners sit 16 slots apart within one 32-partition
    lane group (single DVE stream_shuffle out of PSUM), cos and sin
    multiply (sin in place on the shuffle), one add.
  - Causality at 128-column granularity: diagonal score tiles compute
    only the valid q sub-range; masking is a 0/1 bf16 multiply on 128
    columns of the exp'd probs.  Diagonal tiles with w <= 256 pack both
    heads' scores contiguously inside one PSUM bank so their exp is a
    single ACT instruction too.
  - V transposed to k-major via DMA-transpose; warm-up matmuls ramp the
    PE p-state while the first DMAs land.

Modeled exec time 262.6us (baseline 277.3us); HW rel-err vs the fp32
reference 7.2e-3, verified deterministic across fresh-device runs and
race-clean under CoreSim.
"""

import itertools
import sys

sys.path.insert(0, "/opt/trn_rl_repo")

import numpy as np
import ml_dtypes

import concourse.bass as bass
import concourse.tile as tile
from concourse import bacc
from concourse import mybir
from concourse import bass_utils
from concourse.masks import make_identity

F32 = mybir.dt.float32
F32R = mybir.dt.float32r
BF16 = mybir.dt.bfloat16
F8 = mybir.dt.float8e4
E4 = ml_dtypes.float8_e4m3
AF = mybir.ActivationFunctionType
WSCALE = 64.0  # host pre-scale on all weights (fp8 normal range)

B, T, C, DH = 2, 2048, 2048, 64
NCORE = 8


def _r(ap):
    return ap.bitcast(F32R)


_ROPE_MASK = list(range(16, 32)) + list(range(0, 16))


def _roundrobin(gens):
    pending = [iter(g) for g in gens]
    while pending:
        alive = []
        for g in pending:
            u = next(g, None)
            if u is not None:
                yield u
                alive.append(g)
        pending = alive


class _Kern:
    def __init__(self, tc, io):
        self.tc = tc
        self.nc = tc.nc
        (self.x8T, self.wq8T, self.wkv8T, self.wo8T, self.csT,
         self.bmaskT, self.out) = io
        self.xts = {}

    # ---------- DMA issue helpers ----------
    def _x_dma_half(self, j, half, eng):
        # dram x8T [C, 2*T]: rows = features, col = half*T + t
        # (half 0 = x8, half 1 = dx8).  One tile per j block,
        # col = half*8192 + c*512 + t: ONE 3-dim DMA per half covers the
        # whole block (16 contraction chunks).
        t = self.xts[j]
        src = self.x8T[:, half * T + j * 512: half * T + (j + 1) * 512]
        eng.dma_start(out=t[:, half * 8192:(half + 1) * 8192],
                      in_=src.rearrange("(cx p) t -> p cx t", p=128))

    def issue_x(self, j):
        # SP queue: otherwise idle, so a blocked slot-reuse wait cannot
        # stall inline compute behind it (PE/Pool queues are in-order).
        nc = self.nc
        self.xts[j] = self.xp.tile([128, 16384], F8, tag="x",
                                   name=f"x{j}")
        self._x_dma_half(j, 0, nc.sync)
        self._x_dma_half(j, 1, nc.sync)

    # ---------- generators of PE work units ----------
    def gen_A(self, jn, chains=(4, 0, 5, 1, 2, 3)):
        """QKV projections + RoPE + V transpose for block jn (fp8 DoubleRow
        with error compensation: W8.x8 + (dW8.x8 + W8.dx8) per chunk).
        Yields one callable per PE instruction; non-PE work is emitted
        inline at chain boundaries."""
        nc = self.nc
        DR = mybir.MatmulPerfMode.DoubleRow
        for g in chains:  # default order: k, q0, v, q1, q2, q3
            acc = self.ps.tile([128, 512], F32, tag="accA", bufs=2,
                               name=f"accA{jn}_{g}")
            # weight view [128, nchunk(x), 2(half: 0=dW8 1=W8), 128]
            if g < 4:
                wview, cbase = self.wq8v, g * 16
            elif g == 4:
                wview, cbase = self.wkv8v, 0
            else:
                wview, cbase = self.wkv8v, 16
            # 24 accumulating matmuls, grouped per x tile (cc): 2 main
            # chunk-pairs then 4 residual chunks.
            units = []
            for cc in range(4):
                for cp in (2 * cc, 2 * cc + 1):
                    units.append(("m", cc, cp))
                for c in range(4 * cc, 4 * cc + 4):
                    units.append(("r", cc, c))
            n_u = len(units)
            for idx, (kind, cc, c) in enumerate(units):
                def mk(acc=acc, wview=wview, cbase=cbase, kind=kind, cc=cc,
                       c=c, idx=idx):
                    xv = self.xts[jn][:].rearrange(
                        "p (two cx t) -> p two cx t", two=2, cx=16)
                    if kind == "m":
                        # main: W8 pair over chunks (2c, 2c+1) x x8 pair
                        lh = wview[:, cbase + 2 * c: cbase + 2 * c + 2, 1, :]
                        rh = xv[:, 0, 2 * c: 2 * c + 2, :]
                    else:
                        # residual: dW8[c].x8[c] + W8[c].dx8[c]
                        lh = wview[:, cbase + c, :, :]
                        rh = xv[:, :, c, :]
                    nc.tensor.matmul(acc[:], lhsT=lh, rhs=rh,
                                     start=(idx == 0), stop=(idx == n_u - 1),
                                     perf_mode=DR)
                yield mk
            if g == 5:
                vraw = self.miscp.tile([128, 512], BF16, tag="vraw", bufs=2,
                                       name=f"vraw{jn}")
                nc.vector.tensor_copy(vraw[:], acc[:])
                for tt in range(4):
                    gt = jn * 4 + tt
                    vtmp = self.miscp.tile([128, 128], BF16, tag="vtmp",
                                           bufs=6, name=f"vtmp{gt}")
                    nc.sync.dma_start(out=vtmp[:],
                                      in_=vraw[:, tt * 128:(tt + 1) * 128],
                                      transpose=True)
                    nc.gpsimd.tensor_copy(
                        self.v_aug[:, gt * 65: gt * 65 + 64],
                        vtmp[:, 0:64])
                    nc.gpsimd.tensor_copy(
                        self.v_aug[:, 1040 + gt * 65: 1040 + gt * 65 + 64],
                        vtmp[:, 64:128])
            else:
                # RoPE for a q (g<4) or k (g==4) chain.  The host lays the
                # dh axis out so each rotate-half partner sits 16 positions
                # away within the same 32-partition lane group; the swap is
                # then a single DVE stream_shuffle straight out of PSUM.
                sh = self.ropep.tile([128, 512], F32, tag="sh", bufs=2,
                                     name=f"sh{jn}_{g}")
                nc.vector.stream_shuffle(sh[:], acc[:], _ROPE_MASK)
                jc = slice(jn * 512, (jn + 1) * 512)
                # qT layout: col = j*2048 + d*512 + t so a (d0, d0+1) head
                # pair of one j block is contiguous for the merged score mm
                dst = (self.qT[:, jn * 2048 + g * 512:
                               jn * 2048 + (g + 1) * 512]
                       if g < 4 else self.kT[:, jc])
                tmpc = self.ropep.tile([128, 512], F32, tag="tmpc", bufs=2,
                                       name=f"tmpc{jn}_{g}")
                nc.vector.tensor_mul(
                    tmpc[:], acc[:],
                    self.cs_sb[:, jn * 1024:jn * 1024 + 512])
                # sin multiply in place on the shuffle tile
                nc.vector.tensor_mul(
                    sh[:], sh[:],
                    self.cs_sb[:, jn * 1024 + 512:(jn + 1) * 1024])
                nc.vector.tensor_add(dst, tmpc[:], sh[:])

    def gen_D(self, jo, tag="accD", epilogue=False):
        """Output projection for t-block jo (needs a8/da8 of block jo).
        fp8 DoubleRow: a8 f-pairs x wo8 f-pairs (2 main) then per-f
        residual (a8.dwo8 + da8.wo8) (4)."""
        nc = self.nc
        DR = mybir.MatmulPerfMode.DoubleRow
        # main f(0,1) and the f0/f1 residuals only need heads d=0/1,
        # whose a8/da8 land two attention groups before d=2/3's
        units_def = [("m", 0), ("r", 0), ("r", 1), ("m", 2),
                     ("r", 2), ("r", 3)]

        def mm(acc_ap, tt, cb, kind, f, idx):
            ts = slice(tt * 128, tt * 128 + 128)
            cs = slice(cb * 512, (cb + 1) * 512)
            if kind == "m":
                lh = self.a8v[:, f:f + 2, 0, ts]
                rh = self.wo8v[:, f:f + 2, 1, cs]
            else:
                lh = self.a8v[:, f, :, ts]
                rh = self.wo8v[:, f, :, cs]
            nc.tensor.matmul(acc_ap, lhsT=lh, rhs=rh,
                             start=(idx == 0), stop=(idx == 5),
                             perf_mode=DR)

        for tt in range(jo * 4, jo * 4 + 4):
            ost = self.miscp.tile([128, 2048], BF16, tag="ost", bufs=3,
                                  name=f"ost{tt}")
            if epilogue:
                # pairwise-interleave the cb groups on the halves of a wide
                # sc-tag PSUM tile (bufs=2 -> 4 groups in flight), so each
                # ring slot has two full groups of matmul time to drain
                # through its copies
                for cb0 in (0, 2):
                    acc2 = self.ps.tile([128, 1024], F32, tag="sc", bufs=2,
                                        name=f"od{tt}_{cb0}")
                    for idx, (kind, f) in enumerate(units_def):
                        for k in (0, 1):
                            def mk(acc2=acc2, k=k, tt=tt, cb=cb0 + k,
                                   kind=kind, f=f, idx=idx):
                                mm(acc2[:, k * 512:(k + 1) * 512],
                                   tt, cb, kind, f, idx)
                            yield mk
                    for k in (0, 1):
                        cb = cb0 + k
                        if k == 0:
                            nc.scalar.activation(
                                ost[:, cb * 512:(cb + 1) * 512],
                                acc2[:, 0:512], AF.Copy, scale=1.0)
                        else:
                            nc.vector.tensor_copy(
                                ost[:, cb * 512:(cb + 1) * 512],
                                acc2[:, 512:1024])
                        if tt >= 14:
                            nc.sync.dma_start(
                                out=self.out[tt * 128:(tt + 1) * 128,
                                             cb * 512:(cb + 1) * 512],
                                in_=ost[:, cb * 512:(cb + 1) * 512])
                    if tt < 14 and cb0 == 2:
                        for h in (0, 1):
                            nc.sync.dma_start(
                                out=self.out[tt * 128:(tt + 1) * 128,
                                             h * 1024:(h + 1) * 1024],
                                in_=ost[:, h * 1024:(h + 1) * 1024])
                continue
            for cb in range(4):
                acc = self.ps.tile([128, 512], F32, tag=tag, bufs=2,
                                   name=f"od{tt}_{cb}")
                for idx, (kind, f) in enumerate(units_def):
                    def mk(acc=acc, tt=tt, cb=cb, kind=kind, f=f, idx=idx):
                        mm(acc[:], tt, cb, kind, f, idx)
                    yield mk
                if epilogue and cb % 2 == 0:
                    # alternate the PSUM evacuations between ACT and DVE in
                    # the epilogue so neither serializes the accD ring
                    nc.scalar.activation(ost[:, cb * 512:(cb + 1) * 512],
                                         acc[:], AF.Copy, scale=1.0)
                else:
                    nc.vector.tensor_copy(ost[:, cb * 512:(cb + 1) * 512],
                                          acc[:])
                if epilogue and tt >= 14:
                    # pipeline the very last stores per 512 columns so the
                    # final DMA starts as early as possible
                    nc.sync.dma_start(
                        out=self.out[tt * 128:(tt + 1) * 128,
                                     cb * 512:(cb + 1) * 512],
                        in_=ost[:, cb * 512:(cb + 1) * 512])
                elif cb % 2:
                    h = cb // 2
                    nc.sync.dma_start(
                        out=self.out[tt * 128:(tt + 1) * 128,
                                     h * 1024:(h + 1) * 1024],
                        in_=ost[:, h * 1024:(h + 1) * 1024])

    # ---------- attention over one j block ----------
    def run_C(self, j, fills):
        nc = self.nc
        nk = 4 * j + 4

        def pull(k):
            for _ in range(k):
                u = next(fills, None)
                if u is not None:
                    u()

        for hv, d0 in ((0, 0), (1, 0), (0, 2), (1, 2)):
            pav = self.ps.tile([65, 1024], F32, tag="pav", bufs=1,
                               name=f"pav{j}_{hv}_{d0}")

            def score_exp(i):
                """Scores + exp (+ causal mask) for key tile i; returns the
                prob tile.  sc/pr are [128, 2*512] double tiles covering
                the head pair (d0, d0+1)."""
                m = i - 4 * j
                # causal: for diagonal tiles only q columns >= 128*m can
                # see k tile i; bf16 matmuls run 1 cy/row at any width.
                q0 = max(m, 0) * 128
                sc = self.ps.tile([128, 1024], F32, tag="sc", bufs=2,
                                  name=f"sc{j}_{hv}_{d0}_{i}")
                pr = self.probsp.tile([128, 1024], BF16, tag="pr", bufs=3,
                                      name=f"pr{j}_{hv}_{d0}_{i}")
                kTs = self.kT[hv * 64:hv * 64 + 64, i * 128:(i + 1) * 128]
                w = 512 - q0
                # matmul outputs may not cross a PSUM bank (512 f32):
                # scores stay per-head; the exp merges into one ACT
                # instruction over both heads wherever the two valid
                # ranges can be made contiguous: always for m <= 0, and
                # for m >= 2 by packing h1's scores at sc[w:2w] (both
                # halves then fit inside PSUM bank 0).
                packed = m >= 2
                for h in (0, 1):
                    o = h * w if packed else h * 512 + q0
                    nc.tensor.matmul(
                        sc[:, o:o + w], lhsT=kTs,
                        rhs=self.qT[hv * 64:hv * 64 + 64,
                                    j * 2048 + (d0 + h) * 512 + q0:
                                    j * 2048 + (d0 + h + 1) * 512],
                        start=True, stop=True)
                if m <= 0 or packed:
                    nc.scalar.activation(pr[:, 0:2 * w] if packed else pr[:],
                                         sc[:, 0:2 * w] if packed else sc[:],
                                         AF.Exp, scale=0.125)
                else:  # m == 1: ranges [128:512] and [640:1024]: two exps
                    for h in (0, 1):
                        nc.scalar.activation(
                            pr[:, h * 512 + q0:(h + 1) * 512],
                            sc[:, h * 512 + q0:(h + 1) * 512],
                            AF.Exp, scale=0.125)
                if m >= 0:
                    # only the leading 128 columns of each head's valid
                    # range are partially masked (the triangle)
                    for h in (0, 1):
                        o = h * w if packed else h * 512 + q0
                        nc.vector.tensor_mul(
                            pr[:, o:o + 128], pr[:, o:o + 128],
                            self.bmask[:, m * 512 + q0:m * 512 + q0 + 128])
                return pr

            def att_v(i, pr):
                m = i - 4 * j
                q0 = max(m, 0) * 128
                w = 512 - q0
                packed = m >= 2
                vas = self.v_aug[:, hv * 1040 + i * 65:
                                 hv * 1040 + i * 65 + 65]
                for h in (0, 1):
                    o = h * w if packed else h * 512 + q0
                    nc.tensor.matmul(
                        pav[:, h * 512 + q0:(h + 1) * 512], lhsT=vas,
                        rhs=pr[:, o:o + w],
                        start=(i == 0), stop=(i == nk - 1),
                        skip_group_check=True)

            for i in range(nk):
                pr = score_exp(i)
                pull(6 if j == 0 else 2)
                att_v(i, pr)
            # evacuate pav to SBUF at once (frees the PSUM accumulator for
            # the next group), then normalize from the SBUF copy.
            pnum = self.miscp.tile([65, 1024], F32, tag="pnum", bufs=2,
                                   name=f"pnum{j}_{hv}_{d0}")
            # split across ACT and DVE so both halves evacuate in parallel
            nc.scalar.activation(pnum[:, 0:512], pav[:, 0:512], AF.Copy,
                                 scale=1.0)
            nc.vector.tensor_copy(pnum[:, 512:1024], pav[:, 512:1024])
            bc = self.miscp.tile([64, 1024], F32, tag="bc", bufs=2,
                                 name=f"bc{j}_{hv}_{d0}")
            for h in (0, 1):
                den = self.miscp.tile([1, 512], F32, tag="den", bufs=2,
                                      name=f"den{j}_{hv}_{d0}_{h}")
                nc.vector.reciprocal(den[:], pnum[64:65,
                                                  h * 512:(h + 1) * 512])
                nc.gpsimd.partition_broadcast(bc[:, h * 512:(h + 1) * 512],
                                              den[:])
            if j == 0:
                # drain past the generator tail: a chain's RoPE is emitted
                # by the next() AFTER its last matmul unit, so the deferred
                # q2/q3 chains need pulls beyond their 48 units before
                # group (0,2) reads their qT output
                pull(4)
            for h in (0, 1):
                d = d0 + h
                aTs = self.aT[hv * 64:hv * 64 + 64,
                              d * 2048 + j * 512: d * 2048 + (j + 1) * 512]
                nc.vector.tensor_mul(
                    aTs,
                    pnum[0:64, h * 512:(h + 1) * 512],
                    bc[:, h * 512:(h + 1) * 512])
                # fp8 split for the output projection: a8 = fp8(aT),
                # da8 = aT - a8
                a8s = self.a8c[hv * 64:hv * 64 + 64,
                               d * 4096 + j * 512: d * 4096 + (j + 1) * 512]
                da8s = self.a8c[hv * 64:hv * 64 + 64,
                                d * 4096 + 2048 + j * 512:
                                d * 4096 + 2048 + (j + 1) * 512]
                nc.gpsimd.tensor_copy(a8s, aTs)
                nc.gpsimd.tensor_tensor(out=da8s, in0=aTs, in1=a8s,
                                        op=mybir.AluOpType.subtract)
            pull(8)

    # ---------- full kernel ----------
    def build(self):
        nc = self.nc
        tc = self.tc
        with tc.tile_pool(name="cst", bufs=1) as cst, \
             tc.tile_pool(name="xp", bufs=3) as self.xp, \
             tc.tile_pool(name="ropep", bufs=2) as self.ropep, \
             tc.tile_pool(name="probsp", bufs=4) as self.probsp, \
             tc.tile_pool(name="miscp", bufs=2) as self.miscp, \
             tc.tile_pool(name="ps", bufs=1, space="PSUM") as self.ps:
            # fp8 tiles: interleaved (residual, main) halves per chunk
            #   wq8  [128, 4 chains * 16 chunks * 2 * 128]
            #   wkv8 [128, 2 chains * 16 chunks * 2 * 128]
            #   wo8  [128, 4 fblocks * 2 * 2048]   (half: 0=dwo8 1=wo8)
            #   a8c  [128, 4 dheads * 2 * 2048]    (half: 0=a8 1=da8)
            self.wq8 = cst.tile([128, 16384], F8, name="wq8")
            self.wkv8 = cst.tile([128, 8192], F8, name="wkv8")
            self.qT = cst.tile([128, 4 * 2048], BF16, name="qT")
            self.kT = cst.tile([128, 2048], BF16, name="kT")
            self.v_aug = cst.tile([128, 2 * 16 * 65], BF16, name="v_aug")
            self.aT = cst.tile([128, 4 * 2048], BF16, name="aT")
            self.a8c = cst.tile([128, 16384], F8, name="a8c")
            self.wo8 = cst.tile([128, 16384], F8, name="wo8")
            # col = q*1024 + {0:cos 1:sin}*512 + t  (one DMA per q block)
            self.cs_sb = cst.tile([128, 2 * T], BF16, name="cs_sb")
            self.bmask = cst.tile([128, 2048], BF16, name="bmask")
            self.ident = cst.tile([128, 128], F32, name="ident")
            warm = cst.tile([128, 256], BF16, name="warm")

            # rearranged views used by the matmul generators
            self.wq8v = self.wq8[:].rearrange(
                "p (x two f) -> p x two f", two=2, f=128)
            self.wkv8v = self.wkv8[:].rearrange(
                "p (x two f) -> p x two f", two=2, f=128)
            self.wo8v = self.wo8[:].rearrange(
                "p (f two n) -> p f two n", two=2, n=2048)
            self.a8v = self.a8c[:].rearrange(
                "p (d two t) -> p d two t", two=2, t=2048)

            # --- prologue: small compute + all early DMAs ---
            nc.vector.memset(warm[:], 0.0)
            make_identity(nc, self.ident[:])
            for hv in range(2):
                for gt in range(16):
                    o = hv * 1040 + gt * 65 + 64
                    # v_aug holds 64*v (weights are host-scaled by 64), so
                    # the denominator column must be 64 as well for the
                    # normalize to cancel the scale.
                    nc.vector.memset(self.v_aug[:, o:o + 1], 64.0)

            # weight/x DMAs ordered by first use; weights are stored
            # host-side in chain-major fp8 layout so ONE DMA unblocks a
            # chain.
            def wkv_dma(h):
                nc.sync.dma_start(
                    out=self.wkv8[:, h * 4096:(h + 1) * 4096],
                    in_=self.wkv8T[:, h * 4096:(h + 1) * 4096])
            def wq_dma(g):
                nc.sync.dma_start(
                    out=self.wq8[:, g * 4096:(g + 1) * 4096],
                    in_=self.wq8T[:, g * 4096:(g + 1) * 4096])
            def cs_dma(q, eng=None):
                eng = eng or nc.scalar
                sl = slice(q * 1024, (q + 1) * 1024)
                eng.dma_start(out=self.cs_sb[:, sl], in_=self.csT[:, sl])

            self.xts[0] = self.xp.tile([128, 16384], F8, tag="x",
                                       name="x0")
            wkv_dma(0)
            self._x_dma_half(0, 0, nc.sync)   # all x8 (main) chunks
            wq_dma(0)
            self._x_dma_half(0, 1, nc.sync)   # all dx8 (residual) chunks
            cs_dma(0, eng=nc.sync)
            wkv_dma(1)
            nc.sync.dma_start(out=self.bmask[:], in_=self.bmaskT[:])
            wq_dma(1)
            nc.sync.dma_start(out=self.wq8[:, 8192:16384],
                              in_=self.wq8T[:, 8192:16384])

            # warm-up matmuls: ramp the PE while DMAs land (bf16, cheap,
            # no dependency beyond the DVE memset)
            for w in range(6):
                wacc = self.ps.tile([128, 512], F32, tag="accA", bufs=2,
                                    name=f"warm{w}")
                nc.tensor.matmul(wacc[:, 0:256], lhsT=warm[:, 0:128],
                                 rhs=warm[:], start=True, stop=True)

            # x(1) already streams in while block-0 projections run
            self.issue_x(1)

            # block-0 projections for the first two attention groups
            # (chains q2/q3 become fills inside run_C(0) below, so the
            # attention can start before the wq2/wq3 DMAs have landed)
            for u in self.gen_A(0, chains=(4, 0, 5, 1)):
                u()
            rest0 = self.gen_A(0, chains=(2, 3))

            # --- main pipeline over j ---
            for j in range(4):
                if j == 0:
                    for q in range(1, 4):
                        cs_dma(q)
                if j == 1:
                    # keep the big wo transfer out of the prologue's
                    # critical x/weight window: the DMA must wait for a
                    # read of its target region whose other operand only
                    # becomes ready once the x(0) stream has landed
                    gd = self.miscp.tile([1, 1], F32, tag="gate", bufs=1,
                                         name="wo_gate")
                    # initialize the probe element so the read is defined
                    nc.gpsimd.memset(self.wo8[0:1, 0:1], 0.0)
                    nc.gpsimd.tensor_tensor(
                        out=gd[:], in0=self.wo8[0:1, 0:1],
                        in1=self.xts[0][0:1, 0:1],
                        op=mybir.AluOpType.add)
                    nc.scalar.dma_start(out=self.wo8[:], in_=self.wo8T)
                if j < 2:
                    self.issue_x(j + 2)
                gens = []
                if j == 3:
                    gens.append(itertools.chain(
                        self.gen_D(0, tag="accA"),
                        self.gen_D(1, tag="accA"),
                        self.gen_D(2, tag="accA")))
                if j < 3:
                    gens.append(self.gen_A(j + 1))
                fills = _roundrobin(gens)
                if j == 0:
                    # q2/q3 of block 0 MUST fully drain before group (0,2)
                    # is emitted (their RoPE feeds its scores, and a queued
                    # score matmul would sit ahead of them in the in-order
                    # PE stream): 2 groups x (4 i x pull(4) + pull(8)) = 48
                    # = len(rest0) exactly.
                    fills = itertools.chain(rest0, fills)
                self.run_C(j, fills)
                for u in fills:
                    u()

            # epilogue: last output-projection block
            for u in self.gen_D(3, tag="accA", epilogue=True):
                u()


_cached_nc = None


def _build():
    global _cached_nc
    if _cached_nc is not None:
        return _cached_nc
    nc = bacc.Bacc("TRN2", target_bir_lowering=False, debug=False,
                   num_devices=NCORE)
    io = (
        nc.dram_tensor("x8T", [C, 2 * T], F8, kind="ExternalInput").ap(),
        nc.dram_tensor("wq8T", [128, 16384], F8, kind="ExternalInput").ap(),
        nc.dram_tensor("wkv8T", [128, 8192], F8, kind="ExternalInput").ap(),
        nc.dram_tensor("wo8T", [128, 16384], F8, kind="ExternalInput").ap(),
        nc.dram_tensor("csT", [128, 2 * T], BF16, kind="ExternalInput").ap(),
        nc.dram_tensor("bmaskT", [128, 2048], BF16,
                       kind="ExternalInput").ap(),
        nc.dram_tensor("out", [T, C], BF16, kind="ExternalOutput").ap(),
    )
    with tile.TileContext(nc) as tc:
        with nc.allow_low_precision(reason="bf16 attention operands"):
            _Kern(tc, io).build()
    nc.compile()
    _cached_nc = nc
    return nc


def _prep_in_maps(x, cos, sin, Wq, Wkv, Wo):
    x = np.asarray(x, np.float32)
    cos = np.asarray(cos, np.float32)
    sin = np.asarray(sin, np.float32)
    Wq = np.asarray(Wq, np.float32)
    Wkv = np.asarray(Wkv, np.float32)
    Wo = np.asarray(Wo, np.float32)

    p = np.arange(128)
    # dh layout within each 64-wide head: rotate-half partners (dh, dh+32)
    # are placed 16 apart inside one 32-partition lane group, so the swap
    # is a DVE stream_shuffle.  dhmap[b] = original dh stored at slot b.
    b = np.arange(64)
    dhmap = np.where(b < 16, b,
                     np.where(b < 32, b + 16,
                              np.where(b < 48, b - 16, b)))
    p_dh = dhmap[p % 64]
    # tables carry 1/WSCALE to cancel the host weight scaling on q/k;
    # cos/sin interleaved per 512-block: col = q*1024 + half*512 + t
    cosT = (cos[:, p_dh % 32].T / WSCALE).astype(ml_dtypes.bfloat16)
    sgn = np.where(p_dh < 32, -1.0, 1.0).astype(np.float32)
    sinT = (sin[:, p_dh % 32].T * sgn[:, None] / WSCALE).astype(
        ml_dtypes.bfloat16)
    csT = np.ascontiguousarray(
        np.stack([cosT.reshape(128, 4, 512), sinT.reshape(128, 4, 512)],
                 axis=2).reshape(128, 2 * T))
    n = np.arange(512)
    bmaskT = np.empty((128, 2048), np.float32)
    for m in range(4):
        bmaskT[:, m * 512:(m + 1) * 512] = (
            (128 * m + p)[:, None] <= n[None, :]).astype(np.float32)
    bmaskT = bmaskT.astype(ml_dtypes.bfloat16)

    qperm = np.empty(512, np.int64)
    operm = np.empty(512, np.int64)
    for dd_t in range(4):
        for o in (0, 64):
            hq = dd_t + (o // 64) * 4
            qperm[dd_t * 128 + o: dd_t * 128 + o + 64] = hq * 64 + dhmap
            operm[dd_t * 128 + o: dd_t * 128 + o + 64] = \
                np.arange(hq * 64, hq * 64 + 64)

    def split8(a):
        """fp8 main + residual of a float32 array."""
        m = a.astype(E4)
        r = (a - m.astype(np.float32)).astype(E4)
        return m, r

    in_maps = []
    for b in range(B):
        xb = np.ascontiguousarray(x[b].T)                   # [C, T] f32
        x8, dx8 = split8(xb)
        x8T = np.ascontiguousarray(
            np.stack([x8, dx8], axis=1).reshape(C, 2 * T))  # [C, 2T]
        for g in range(4):
            # --- Wq: [p, gg, c, half(0=dW8 1=W8), f] -> [128, 16384] ---
            wqT0 = Wq[g * 512:(g + 1) * 512, :][qperm].T * WSCALE
            q8, d8 = split8(wqT0)                           # [2048, 512]
            wq8T = np.ascontiguousarray(
                np.stack([d8, q8], 0).reshape(2, 16, 128, 4, 128)
                .transpose(2, 3, 1, 0, 4).reshape(128, 16384))
            # --- Wkv ---
            krows = Wkv[128 * g:128 * g + 128]
            kperm = np.concatenate([dhmap, 64 + dhmap])
            wkvT0 = np.concatenate(
                [krows[kperm],
                 Wkv[512 + 128 * g:512 + 128 * g + 128]], 0).T * WSCALE
            q8, d8 = split8(wkvT0)                          # [2048, 256]
            wkv8T = np.ascontiguousarray(
                np.stack([d8, q8], 0).reshape(2, 16, 128, 2, 128)
                .transpose(2, 3, 1, 0, 4).reshape(128, 8192))
            # --- Wo: [p, f, half(0=dwo8 1=wo8), n] -> [128, 16384] ---
            woT0 = Wo[:, g * 512:(g + 1) * 512].T[operm] * WSCALE
            q8, d8 = split8(woT0)                           # [512, 2048]
            wo8T = np.ascontiguousarray(
                np.stack([d8, q8], 0).reshape(2, 4, 128, 2048)
                .transpose(2, 1, 0, 3).reshape(128, 16384))
            in_maps.append({"x8T": x8T, "wq8T": wq8T, "wkv8T": wkv8T,
                            "wo8T": wo8T, "csT": csT,
                            "bmaskT": bmaskT})
    return in_maps


def _run(x, cos, sin, Wq, Wkv, Wo, trace=False):
    nc = _build()
    in_maps = _prep_in_maps(x, cos, sin, Wq, Wkv, Wo)
    res = bass_utils.run_bass_kernel_spmd(nc, in_maps,
                                          core_ids=list(range(NCORE)),
                                          trace=trace)
    out = np.zeros((B, T, C), np.float32)
    for b in range(B):
        for g in range(4):
            out[b] += res.results[b * 4 + g]["out"].astype(np.float32)
    out *= 1.0 / WSCALE  # weights are host-scaled by WSCALE
    return out, res


def kernel(x, cos, sin, Wq, Wkv, Wo):
    out, _ = _run(x, cos, sin, Wq, Wkv, Wo)
    return out

